# revision 1
# baseline (speedup 1.0000x reference)
"""Trainium2 Bass kernel for nn_CurrentHypTokenRefiner (pooling, B=256, N=2048, Dh=128).

Pure batch data-parallel over 8 NeuronCores (32 batches each).  Each
batch's demo tokens (1MB fp32) are read from HBM exactly once, converted
to bf16, kept resident in SBUF in both [tok, d] and [d, tok] layouts
(the latter via the DMA xbar transpose engine), and all three
attention-pool passes run on-chip.  Per-batch scalar chains run as
[8, 128] fp32 row tiles batched across a group of 8 batches.
"""
import numpy as np

import concourse.bacc as bacc
import concourse.bass as bass
import concourse.tile as tile
from concourse import mybir
from concourse.masks import make_identity
from concourse.bass_utils import run_bass_kernel_spmd

FP32 = mybir.dt.float32
BF16 = mybir.dt.bfloat16
AF = mybir.ActivationFunctionType
OP = mybir.AluOpType

N_CORES = 8
B, N, DH, DG, HID = 256, 2048, 128, 256, 256
BC = B // N_CORES          # batches per core = 32
G = 8                      # batches per resident group
NT = N // 128              # 16 token tiles per batch
SCALE = 1.0 / (DH ** 0.5)
LN_EPS = 1e-5
import os
_USE_APPROX_RECIP = os.environ.get("KAPPROX", "1") == "1"
_TRANS_MODE = os.environ.get("KTRANS", "dma")
_KSTAGE = int(os.environ.get("KSTAGE", "99"))

WEIGHT_NAMES = ["seed_g_w", "seed_g_b", "seed_d_w", "pool_wq_w", "pool_wk_w",
                "pool_wv_w", "q_u_w", "q_g_w", "q_g_b", "kv_w", "mlp_w1",
                "mlp_b1", "mlp_w2", "mlp_b2", "ln_g", "ln_b"]

_cache = {}


def _build(bc):
    ng = bc // G
    nc = bacc.Bacc("TRN2", target_bir_lowering=False, debug=False,
                   num_devices=N_CORES)

    demo_d = nc.dram_tensor("demo", [bc, N, DH], FP32, kind="ExternalInput").ap()
    rho_d = nc.dram_tensor("rho", [bc, DG], FP32, kind="ExternalInput").ap()
    w_d = {}
    for name, shape in [
        ("seed_g_w", [DH, DG]), ("seed_g_b", [DH]), ("seed_d_w", [DH, DH]),
        ("pool_wq_w", [DH, DH]), ("pool_wk_w", [DH, DH]), ("pool_wv_w", [DH, DH]),
        ("q_u_w", [DH, DH]), ("q_g_w", [DH, DG]), ("q_g_b", [DH]),
        ("kv_w", [DH, DH]), ("mlp_w1", [HID, DH]), ("mlp_b1", [HID]),
        ("mlp_w2", [DH, HID]), ("mlp_b2", [DH]), ("ln_g", [DH]), ("ln_b", [DH]),
    ]:
        w_d[name] = nc.dram_tensor(name, shape, FP32, kind="ExternalInput").ap()
    u_out_d = nc.dram_tensor("u_out", [bc, DH], FP32, kind="ExternalOutput").ap()

    with tile.TileContext(nc) as tc:
        _body(nc, tc, demo_d, rho_d, w_d, u_out_d, ng)
    nc.compile()
    return nc


def _body(nc, tc, demo_d, rho_d, w_d, u_out_d, ng):
    import contextlib
    ctx = contextlib.ExitStack()

    consts = ctx.enter_context(tc.tile_pool(name="consts", bufs=1))
    wpool = ctx.enter_context(tc.tile_pool(name="wpool", bufs=1))
    xbf_p = ctx.enter_context(tc.tile_pool(name="xbfp", bufs=G))
    xt_p = ctx.enter_context(tc.tile_pool(name="xtp", bufs=G))
    sqt_p = ctx.enter_context(tc.tile_pool(name="sqtp", bufs=2))
    arr_p = ctx.enter_context(tc.tile_pool(name="arrp", bufs=2))
    ssp = ctx.enter_context(tc.tile_pool(name="ssp", bufs=1))
    chain_p = ctx.enter_context(tc.tile_pool(name="chainp", bufs=2))
    tiny_p = ctx.enter_context(tc.tile_pool(name="tinyp", bufs=2))
    sc_ps_p = ctx.enter_context(tc.tile_pool(name="scpsp", bufs=1, space="PSUM"))
    ws_ps_p = ctx.enter_context(tc.tile_pool(name="wspsp", bufs=1, space="PSUM"))
    sm_ps_p = ctx.enter_context(tc.tile_pool(name="smpsp", bufs=2 if _TRANS_MODE == "dma" else 1, space="PSUM"))

    def smps(shape, dtype=FP32):
        return sm_ps_p.tile(shape, dtype, name="smt", tag="sm")

    # ---------- constants ----------
    ident_f = consts.tile([128, 128], FP32)
    make_identity(nc, ident_f)
    ident_b = consts.tile([128, 128], BF16)
    make_identity(nc, ident_b)
    ones_sel = consts.tile([128, G, G], BF16)
    nc.vector.memset(ones_sel, 0.0)
    for b in range(G):
        nc.vector.memset(ones_sel[:, b, b : b + 1], 1.0)
    ones_row = consts.tile([1, G], FP32)
    nc.vector.memset(ones_row, 1.0)
    eps_c = consts.tile([G, 1], FP32)
    nc.vector.memset(eps_c, LN_EPS)

    def bias_col(name):
        t = consts.tile([128, 1], FP32, name=name + "_c", tag=name + "_c")
        nc.sync.dma_start(out=t, in_=w_d[name].rearrange("(d one) -> d one", one=1))
        return t

    seed_g_b_c = bias_col("seed_g_b")
    q_g_b_c = bias_col("q_g_b")

    def row_bcast(name, parts, dim):
        t = consts.tile([parts, dim], FP32, name=name + "_r", tag=name + "_r")
        src = w_d[name]
        nc.sync.dma_start(
            out=t,
            in_=bass.AP(tensor=src.tensor, offset=src.offset,
                        ap=[[0, parts]] + src.ap),
        )
        return t

    ln_g_r = row_bcast("ln_g", G, DH)
    ln_b_r = row_bcast("ln_b", G, DH)
    b1_row = consts.tile([1, HID], FP32)
    nc.sync.dma_start(out=b1_row, in_=w_d["mlp_b1"].rearrange("(one d) -> one d", one=1))
    b2_row = consts.tile([1, DH], FP32)
    nc.sync.dma_start(out=b2_row, in_=w_d["mlp_b2"].rearrange("(one d) -> one d", one=1))

    # ---------- load + transpose weights ----------
    def load_w(name, p, f):
        t = wpool.tile([p, f], FP32, name=name, tag=name)
        nc.sync.dma_start(out=t, in_=w_d[name])
        return t

    seed_g_w = load_w("seed_g_w", DH, DG)
    seed_d_w = load_w("seed_d_w", DH, DH)
    pool_wq_w = load_w("pool_wq_w", DH, DH)
    pool_wk_w = load_w("pool_wk_w", DH, DH)
    pool_wv_w = load_w("pool_wv_w", DH, DH)
    q_u_w = load_w("q_u_w", DH, DH)
    q_g_w = load_w("q_g_w", DH, DG)
    kv_w = load_w("kv_w", DH, DH)
    mlp_w2 = load_w("mlp_w2", DH, HID)
    # mlp_w1 [256, 128] split into two 128-partition halves
    mlp_w1h = wpool.tile([128, 2, DH], FP32, name="mlp_w1h", tag="mlp_w1h")
    for k in range(2):
        nc.sync.dma_start(out=mlp_w1h[:, k, :],
                          in_=w_d["mlp_w1"][128 * k : 128 * (k + 1), :])

    def pe_t(dst, src_ap, np_):
        """dst (sbuf) = src[np_, nf].T via PE."""
        nf = dst.shape[0]
        ps = smps([nf, np_])
        nc.tensor.transpose(ps, src_ap, ident_f[0:np_, 0:np_])
        nc.scalar.copy(dst, ps)

    def transpose_weight(src, name, p, f):
        outs = []
        for k in range(f // 128):
            t = wpool.tile([128, p], FP32, name=f"{name}T{k}", tag=f"{name}T{k}")
            pe_t(t, src[:, 128 * k : 128 * (k + 1)], p)
            outs.append(t)
        return outs

    seed_g_wT = transpose_weight(seed_g_w, "seed_g_w", DH, DG)
    pool_wq_wT = transpose_weight(pool_wq_w, "pool_wq_w", DH, DH)[0]
    seed_d_wT = transpose_weight(seed_d_w, "seed_d_w", DH, DH)[0]
    q_u_wT = transpose_weight(q_u_w, "q_u_w", DH, DH)[0]
    q_g_wT = transpose_weight(q_g_w, "q_g_w", DH, DG)
    kv_wT = transpose_weight(kv_w, "kv_w", DH, DH)[0]
    w2T = transpose_weight(mlp_w2, "mlp_w2", DH, HID)
    w1T = wpool.tile([128, HID], FP32, name="w1T", tag="w1T")
    for k in range(2):
        pe_t(w1T[:, 128 * k : 128 * (k + 1)], mlp_w1h[:, k, :], 128)
    m1 = wpool.tile([128, 128], FP32, name="m1", tag="m1")
    m1_ps = smps([128, 128])
    nc.tensor.matmul(m1_ps, pool_wv_w, seed_d_wT, start=True, stop=True)
    nc.scalar.copy(m1, m1_ps)

    # ---------- prelude: q_seed, q~1 for all batches ----------
    bc = ng * G
    rho_rows = wpool.tile([bc, DG], FP32, name="rho_rows", tag="rho_rows")
    nc.sync.dma_start(out=rho_rows, in_=rho_d)
    rho_T = wpool.tile([128, 2, bc], FP32, name="rho_T", tag="rho_T")
    for k in range(2):
        pe_t(rho_T[:, k, :], rho_rows[:, 128 * k : 128 * (k + 1)], bc)

    qs_ps = smps([128, bc])
    for k in range(2):
        nc.tensor.matmul(qs_ps, seed_g_wT[k], rho_T[:, k, :],
                         start=(k == 0), stop=(k == 1))
    q_seedT = wpool.tile([128, bc], FP32, name="q_seedT", tag="q_seedT")
    nc.scalar.activation(q_seedT, qs_ps, AF.Identity, bias=seed_g_b_c)
    qk_ps = smps([128, bc])
    nc.tensor.matmul(qk_ps, pool_wq_wT, q_seedT, start=True, stop=True)
    qkT = wpool.tile([128, bc], FP32, name="qkT", tag="qkT")
    nc.scalar.copy(qkT, qk_ps)
    q1_ps = smps([128, bc])
    nc.tensor.matmul(q1_ps, pool_wk_w, qkT, start=True, stop=True)
    qt1T_bf = wpool.tile([128, bc], BF16, name="qt1T_bf", tag="qt1T_bf")
    nc.scalar.copy(qt1T_bf, q1_ps)
    qs_rows = []
    for gi in range(ng):
        t = wpool.tile([G, 128], FP32, name=f"qsr{gi}", tag=f"qsr{gi}")
        pe_t(t, q_seedT[:, G * gi : G * (gi + 1)], 128)
        qs_rows.append(t)

    # ---------- [G, *] row-chain helpers ----------
    def tt(shape, name):
        return tiny_p.tile(shape, FP32, name=name, tag=name)

    def ttr_norm2(rows, d):
        scr = chain_p.tile([G, d], FP32, name="n2scr", tag="n2scr")
        nc.vector.tensor_mul(scr, rows, rows)
        n2 = tt([G, 1], "n2t")
        nc.vector.tensor_reduce(out=n2, in_=scr, axis=mybir.AxisListType.X,
                                op=OP.add)
        return n2

    def recip(dst, src):
        nc.vector.reciprocal(dst, src)

    def expmap0_rows(v_rows, name):
        n2 = ttr_norm2(v_rows, DH)
        n_t = tt([G, 1], "emn")
        nc.scalar.activation(n_t, n2, AF.Sqrt)
        nc.vector.tensor_scalar(out=n_t, in0=n_t, scalar1=1e-7, scalar2=None,
                                op0=OP.max)
        th = tt([G, 1], "emth")
        nc.scalar.activation(th, n_t, AF.Tanh)
        ninv = tt([G, 1], "emninv")
        recip(ninv, n_t)
        fac = tt([G, 1], "emfac")
        nc.vector.tensor_tensor(out=fac, in0=th, in1=ninv, op=OP.mult)
        out = chain_p.tile([G, DH], FP32, name=name, tag=name)
        nc.vector.tensor_scalar_mul(out=out, in0=v_rows, scalar1=fac)
        return out

    def logmap0_rows(u_rows, name):
        n2 = ttr_norm2(u_rows, DH)
        n_t = tt([G, 1], "lmn")
        nc.scalar.activation(n_t, n2, AF.Sqrt)
        r = tt([G, 1], "lmr")
        nc.vector.tensor_scalar(out=r, in0=n_t, scalar1=1.0 - 1e-5,
                                scalar2=None, op0=OP.min)
        ln1p = tt([G, 1], "lmp")
        nc.scalar.activation(ln1p, r, AF.Ln, bias=1.0, scale=1.0)
        ln1m = tt([G, 1], "lmm")
        nc.scalar.activation(ln1m, r, AF.Ln, bias=1.0, scale=-1.0)
        at2 = tt([G, 1], "lmat")
        nc.vector.tensor_sub(at2, ln1p, ln1m)
        nc.vector.tensor_scalar(out=n_t, in0=n_t, scalar1=1e-7, scalar2=None,
                                op0=OP.max)
        ninv = tt([G, 1], "lmninv")
        recip(ninv, n_t)
        fac = tt([G, 1], "lmfac")
        nc.vector.tensor_tensor(out=fac, in0=at2, in1=ninv, op=OP.mult)
        nc.vector.tensor_scalar(out=fac, in0=fac, scalar1=0.5, scalar2=None,
                                op0=OP.mult)
        out = chain_p.tile([G, DH], FP32, name=name, tag=name)
        nc.vector.tensor_scalar_mul(out=out, in0=u_rows, scalar1=fac)
        return out

    def transpose_rows(rows, name):
        ps = smps([128, G])
        nc.tensor.transpose(ps, rows, ident_f[0:G, 0:G])
        cols = chain_p.tile([128, G], FP32, name=name, tag=name)
        nc.vector.tensor_copy(cols, ps)
        return cols

    def rows_mm(mms, nf, bias_row=None):
        ps = smps([G, nf])
        nmm = len(mms) + (1 if bias_row is not None else 0)
        for i, (l, r) in enumerate(mms):
            nc.tensor.matmul(ps, l, r, start=(i == 0), stop=(i == nmm - 1))
        if bias_row is not None:
            nc.tensor.matmul(ps, ones_row, bias_row, start=False, stop=True)
        return ps

    # ---------- main loop ----------
    qtT_bf = None
    for gi in range(ng):
        x_bf, x_t = [], []
        n2_ps = sc_ps_p.tile([G, N], FP32, name="scps", tag="sc")
        for b in range(G):
            xa = arr_p.tile([128, NT, 128], FP32, name="xa", tag="xa")
            src = demo_d[G * gi + b].rearrange("(g p) d -> p g d", p=128)
            nc.sync.dma_start(out=xa, in_=src)
            xb = xbf_p.tile([128, NT, 128], BF16, name="xb", tag="xb")
            nc.scalar.copy(out=xb, in_=xa)
            xt = xt_p.tile([128, NT, 128], BF16, name="xt", tag="xt")
            if _TRANS_MODE == "dma":
                nc.sync.dma_start_transpose(
                    out=xt, in_=xb.rearrange("p a b -> p (a b)"))
            else:
                for g in range(NT):
                    tp_ps = sm_ps_p.tile([128, 128], BF16, name="tp_ps", tag="tp")
                    nc.tensor.transpose(tp_ps, xb[:, g, :], ident_b)
                    nc.vector.tensor_copy(xt[:, g, :], tp_ps)
            sq = sqt_p.tile([128, NT, 128], BF16, name="sq", tag="sq")
            nc.vector.tensor_mul(sq, xt, xt)
            for j in range(4):
                nc.tensor.matmul(
                    n2_ps[:, 512 * j : 512 * (j + 1)],
                    ones_sel[:, b, :], sq[:, 4 * j : 4 * j + 4, :],
                    start=(b == 0), stop=(b == G - 1))
            x_bf.append(xb)
            x_t.append(xt)

        if _KSTAGE <= 1:
            dummy = chain_p.tile([G, DH], FP32, name="dummy", tag="dummy")
            nc.vector.tensor_copy(dummy, n2_ps[:, 0:DH])
            nc.sync.dma_start(out=u_out_d[G * gi : G * (gi + 1), :], in_=dummy)
            continue

        # ---- alpha ----
        n_sb = ssp.tile([G, N], FP32, name="ss1", tag="ss1")
        nc.scalar.activation(n_sb, n2_ps, AF.Sqrt)
        invh = ssp.tile([G, N], FP32, name="ss2", tag="ss2")
        if _USE_APPROX_RECIP:
            nc.vector.reciprocal_approx_fast(out=invh, in_=n_sb)
        else:
            nc.vector.reciprocal(out=invh, in_=n_sb)
        r_sb = ssp.tile([G, N], FP32, name="ss3", tag="ss3")
        nc.vector.tensor_scalar(out=r_sb, in0=n_sb, scalar1=1.0 - 1e-5,
                                scalar2=None, op0=OP.min)
        ln1p = ssp.tile([G, N], FP32, name="ss1b", tag="ss1")
        nc.scalar.activation(ln1p, r_sb, AF.Ln, bias=1.0, scale=1.0)
        ln1m = ssp.tile([G, N], FP32, name="ssS", tag="ssS")
        nc.scalar.activation(ln1m, r_sb, AF.Ln, bias=1.0, scale=-1.0)
        at2 = ssp.tile([G, N], FP32, name="ss3b", tag="ss3")
        nc.vector.tensor_sub(at2, ln1p, ln1m)
        alpha = ssp.tile([G, N], BF16, name="alpha", tag="alpha", bufs=2)
        nc.vector.scalar_tensor_tensor(out=alpha, in0=at2, scalar=0.5, in1=invh,
                                       op0=OP.mult, op1=OP.mult)

        if _KSTAGE <= 2:
            dummy = chain_p.tile([G, DH], FP32, name="dummy", tag="dummy")
            nc.vector.tensor_copy(dummy, at2[:, 0:DH])
            nc.sync.dma_start(out=u_out_d[G * gi : G * (gi + 1), :], in_=dummy)
            continue

        u_ln = None
        for k in range(3):
            sel = ssp.tile([128, G, G], BF16, name="sel", tag="sel", bufs=2)
            nc.vector.memset(sel, 0.0)
            qsrc = qt1T_bf[:, G * gi : G * (gi + 1)] if k == 0 else qtT_bf
            for b in range(G):
                nc.vector.tensor_copy(out=sel[:, b, b : b + 1],
                                      in_=qsrc[:, b : b + 1])
            sc_ps = sc_ps_p.tile([G, N], FP32, name="scps", tag="sc")
            for b in range(G):
                for j in range(4):
                    nc.tensor.matmul(
                        sc_ps[:, 512 * j : 512 * (j + 1)],
                        sel[:, b, :], x_t[b][:, 4 * j : 4 * j + 4, :],
                        start=(b == 0), stop=(b == G - 1))
            s_sb = ssp.tile([G, N], FP32, name="ssS2", tag="ssS2")
            nc.vector.tensor_mul(s_sb, sc_ps, alpha)
            e_bf = ssp.tile([G, N], BF16, name="e_bf", tag="e_bf")
            z_t = tt([G, 1], "z_t")
            nc.scalar.activation(out=e_bf, in_=s_sb, func=AF.Exp,
                                 scale=SCALE, accum_out=z_t)
            w_bf = ssp.tile([G, N], BF16, name="w_bf", tag="w_bf")
            nc.vector.tensor_mul(w_bf, e_bf, alpha)
            zr = tt([G, 1], "zr")
            recip(zr, z_t)
            if _KSTAGE <= 3:
                dummy = chain_p.tile([G, DH], FP32, name="dummy", tag="dummy")
                nc.vector.tensor_copy(dummy, s_sb[:, 0:DH])
                nc.sync.dma_start(out=u_out_d[G * gi : G * (gi + 1), :], in_=dummy)
                break

            wt_ps = smps([128, NT, G], BF16)
            for g in range(NT):
                nc.tensor.transpose(wt_ps[:, g, :],
                                    w_bf[:, 128 * g : 128 * (g + 1)],
                                    ident_b[0:G, 0:G])
            w_cols = ssp.tile([128, NT, G], BF16, name="w_cols", tag="w_cols",
                              bufs=2)
            nc.vector.tensor_copy(w_cols, wt_ps)
            ws_ps = ws_ps_p.tile([G, G, 128], FP32, name="wsps", tag="ws")
            for b in range(G):
                for g in range(NT):
                    nc.tensor.matmul(ws_ps[:, b, :], w_cols[:, g, :],
                                     x_bf[b][:, g, :],
                                     start=(g == 0), stop=(g == NT - 1))
            wst = ssp.tile([G, G, 128], FP32, name="wst", tag="wst")
            nc.vector.tensor_copy(wst, ws_ps)
            t_rows = chain_p.tile([G, 128], FP32, name="t_rows", tag="t_rows")
            for b in range(G):
                nc.sync.dma_start(out=t_rows[b : b + 1, :],
                                  in_=wst[b : b + 1, b, :])
            pooled = chain_p.tile([G, 128], FP32, name="pooled", tag="pooled")
            nc.vector.tensor_scalar_mul(out=pooled, in0=t_rows, scalar1=zr)
            if _KSTAGE <= 4:
                nc.sync.dma_start(out=u_out_d[G * gi : G * (gi + 1), :], in_=pooled)
                break

            if k == 0:
                pooled_T = transpose_rows(pooled, "pooled_T")
                v_ps = rows_mm([(pooled_T, m1)], DH)
                v_rows = chain_p.tile([G, DH], FP32, name="v_rows", tag="v_rows")
                nc.vector.tensor_add(v_rows, v_ps, qs_rows[gi])
                if _KSTAGE == 42:
                    nc.sync.dma_start(out=u_out_d[G * gi : G * (gi + 1), :],
                                      in_=v_rows)
                    break
                n2d = ttr_norm2(v_rows, DH)
                if _KSTAGE == 43:
                    dummy2 = chain_p.tile([G, DH], FP32, name="dummy2", tag="dummy2")
                    nc.vector.tensor_scalar_mul(out=dummy2, in0=v_rows, scalar1=n2d)
                    nc.sync.dma_start(out=u_out_d[G * gi : G * (gi + 1), :], in_=dummy2)
                    break
                u_rows = expmap0_rows(v_rows, "u_rows")
            else:
                pooled_T = transpose_rows(pooled, "pooled_T")
                d_ps = rows_mm([(pooled_T, kv_wT)], DH)
                delta = chain_p.tile([G, DH], FP32, name="delta", tag="delta")
                nc.vector.tensor_copy(delta, d_ps)
                delta_T = transpose_rows(delta, "delta_T")
                h_ps = rows_mm([(delta_T, w1T)], HID, bias_row=b1_row)
                h_rows = chain_p.tile([G, HID], FP32, name="h_rows", tag="h_rows")
                nc.scalar.activation(h_rows, h_ps, AF.Gelu)
                hT0 = transpose_rows(h_rows[:, 0:128], "hT0")
                hT1 = transpose_rows(h_rows[:, 128:256], "hT1")
                upd_ps = rows_mm([(hT0, w2T[0]), (hT1, w2T[1])], DH,
                                 bias_row=b2_row)
                u_in = chain_p.tile([G, DH], FP32, name="u_in", tag="u_in")
                nc.vector.tensor_add(u_in, u_ln, upd_ps)
                u_rows = expmap0_rows(u_in, "u_rows")

            if k == 2 or _KSTAGE <= 5:
                nc.sync.dma_start(out=u_out_d[G * gi : G * (gi + 1), :],
                                  in_=u_rows)
                break

            u_tan = logmap0_rows(u_rows, "u_tan")
            stats = tt([G, 6], "stats")
            nc.vector.bn_stats(out=stats, in_=u_tan)
            mv = tt([G, 2], "mv")
            nc.vector.bn_aggr(out=mv, in_=stats)
            rstd = tt([G, 1], "rstd")
            nc.scalar.activation(rstd, mv[:, 1:2], AF.Sqrt, bias=eps_c)
            recip(rstd, rstd)
            u_ln0 = chain_p.tile([G, DH], FP32, name="u_ln0", tag="u_ln0")
            nc.vector.tensor_scalar(out=u_ln0, in0=u_tan, scalar1=mv[:, 0:1],
                                    scalar2=rstd, op0=OP.subtract, op1=OP.mult)
            u_ln = chain_p.tile([G, DH], FP32, name="u_ln", tag="u_ln")
            nc.vector.tensor_mul(u_ln, u_ln0, ln_g_r)
            nc.vector.tensor_add(u_ln, u_ln, ln_b_r)
            u_ln_T = transpose_rows(u_ln, "u_ln_T")
            q_ps = smps([128, G])
            nc.tensor.matmul(q_ps, q_u_wT, u_ln_T, start=True, stop=False)
            for kk in range(2):
                nc.tensor.matmul(q_ps, q_g_wT[kk],
                                 rho_T[:, kk, G * gi : G * (gi + 1)],
                                 start=False, stop=(kk == 1))
            qb_T = chain_p.tile([128, G], FP32, name="qb_T", tag="qb_T")
            nc.scalar.activation(qb_T, q_ps, AF.Identity, bias=q_g_b_c)
            qt_ps = smps([128, G])
            nc.tensor.matmul(qt_ps, kv_w, qb_T, start=True, stop=True)
            qtT_bf = chain_p.tile([128, G], BF16, name="qtT_bf", tag="qtT_bf")
            nc.vector.tensor_copy(qtT_bf, qt_ps)

    ctx.close()


def _get(bc=BC):
    if bc not in _cache:
        _cache[bc] = _build(bc)
    return _cache[bc]


def kernel(**inputs):
    nc = _get()
    demo = np.ascontiguousarray(np.asarray(inputs["demo_hyp"], np.float32))
    rho = np.ascontiguousarray(np.asarray(inputs["rho_g"], np.float32))
    weights = {k: np.ascontiguousarray(np.asarray(inputs[k], np.float32))
               for k in WEIGHT_NAMES}
    in_maps = []
    for c in range(N_CORES):
        m = {"demo": demo[BC * c : BC * (c + 1)],
             "rho": rho[BC * c : BC * (c + 1)]}
        m.update(weights)
        in_maps.append(m)
    res = run_bass_kernel_spmd(nc, in_maps, core_ids=list(range(N_CORES)))
    out = np.concatenate([r["u_out"] for r in res.results], axis=0)
    return out.astype(np.float32)

